# revision 7
# baseline (speedup 1.0000x reference)
"""Trainium2 Bass kernel for nn_CRModule (retrieval_knn).

reference:
    xf = x.reshape(4096, 4096); xa = xf[:, ::2]; xb = xf[:, 1::2]   # [T=4096, 2048]
    sq[i,j] = |xa[:,i]|^2 + |xb[:,j]|^2 - 2 * xa[:,i].xb[:,j]
    wsum = fc_weight.sum(0); wa = wsum[::2]; wb = wsum[1::2]
    scores[i,j] = ((wa[i]+wb[j]) * sqrt(max(sq,0)))**2
                = (wa[i]+wb[j])**2 * max(sq[i,j], 0)     # sqrt cancels

v2 strategy (single SPMD launch, 2x4 output grid):
  Core d (r=d>>2, c=d&3) owns a [1024, 512] block of scores:
    rows  = 1024r + (256(d&3) + li) % 1024   (own na/wa shard first)
    cols  = 512c  + (256r + lj) % 512        (own nb/wb shard first)
  Main matmul (-2a)^T b in fp8 e4m3 with DoubleRow perf mode (0.5 cyc/row).
  fc column sums accumulate on DVE from fp16 (o-tiles innermost), partition
  -reduced by one f32 PE matmul. Norm shards (256 ch each) via ScalarE
  squares + ones-matmuls. One AllGather ships [na|nb|wa|wb] shards (4KB);
  post-gather reads use partition_id-derived dynamic DRAM offsets so one
  compiled program serves all 8 cores. Epilogue fused in PSUM:
  out = max(ps + na + nb, 0) * (wa + wb)^2.
  DMA order: xbn,xan (norm+lhsT shards) -> fcs (12 chunks) -> xar,xbr, so
  the w AllGather latency hides under the trailing x stream.
"""

import numpy as np
import ml_dtypes

import concourse.bass as bass
import concourse.tile as tile
from concourse import bacc, mybir
from concourse.bass_utils import run_bass_kernel_spmd

BF16 = mybir.dt.bfloat16
F32 = mybir.dt.float32
FP16 = mybir.dt.float16
FP8 = mybir.dt.float8e4
NP_BF16 = ml_dtypes.bfloat16
NP_FP8 = ml_dtypes.float8_e4m3
ALU = mybir.AluOpType
DR = mybir.MatmulPerfMode.DoubleRow

D = 8
T = 4096
C = 4096
KT = 32          # 128-row k-tiles
KK = 16          # DoubleRow k-pairs
CA = 2048
MB = 1024        # output rows per core
NBC = 512        # output cols per core
O = 12288
OC = 12          # fc chunks (1024 rows each)

_cache = {}


def _build():
    nc = bacc.Bacc("TRN2", target_bir_lowering=False, debug=False, num_devices=D)
    xan_d = nc.dram_tensor("xan", [128, KT, 256], FP8, kind="ExternalInput").ap()
    xar_d = nc.dram_tensor("xar", [128, KT, 768], FP8, kind="ExternalInput").ap()
    xbn_d = nc.dram_tensor("xbn", [128, KT, 256], FP8, kind="ExternalInput").ap()
    xbr_d = nc.dram_tensor("xbr", [128, KT, 256], FP8, kind="ExternalInput").ap()
    fcs_d = nc.dram_tensor("fcs", [128, OC, 512, 8], FP16, kind="ExternalInput").ap()
    out_d = nc.dram_tensor("scores", [MB, NBC], F32, kind="ExternalOutput").ap()
    pk_in = nc.dram_tensor("pk_in", [1, 1024], F32).ap()
    pk_sh = nc.dram_tensor("pk_sh", [D, 1024], F32, addr_space="Shared").ap()
    grp = [list(range(D))]

    with tile.TileContext(nc) as tc:
        with (
            tc.tile_pool(name="xres", bufs=1) as xres,
            tc.tile_pool(name="fcp", bufs=3) as fcp,
            tc.tile_pool(name="x2p", bufs=2) as x2p,
            tc.tile_pool(name="small", bufs=1) as small,
            tc.tile_pool(name="w2p", bufs=2) as w2p,
            tc.tile_pool(name="outp", bufs=2) as outp,
            tc.tile_pool(name="psmain", bufs=1, space="PSUM") as psmain,
            tc.tile_pool(name="pse", bufs=1, space="PSUM") as pse,
        ):
            # ---- dynamic-offset registers (scalar engine; issued early) ----
            pid = nc.scalar.partition_id()
            r4 = pid & 4            # 4*r  (bit 2)
            cb = pid & 3            # c within r-block
            r1 = pid >> 2           # r

            # ---- DMA emission order = arrival priority ----
            xbn_t, xan_t = [], []
            for g in range(2):
                xb_c = xres.tile([128, 16, 256], FP8, name=f"xbn{g}", tag=f"xbn{g}")
                nc.sync.dma_start(xb_c[:], xbn_d[:, 16 * g:16 * (g + 1), :])
                xbn_t.append(xb_c)
            for g in range(2):
                xa_c = xres.tile([128, 16, 256], FP8, name=f"xan{g}", tag=f"xan{g}")
                nc.sync.dma_start(xa_c[:], xan_d[:, 16 * g:16 * (g + 1), :])
                xan_t.append(xa_c)
            fct = []
            for oc in range(OC):
                f = fcp.tile([128, 512, 8], FP16, name=f"fc{oc}", tag="fc")
                nc.sync.dma_start(f[:], fcs_d[:, oc, :, :])
                fct.append(f)
            xar_t = [None] * 4
            xbr_t = [None] * 2
            for g in (0, 1, 2, 3):
                x_c = xres.tile([128, 8, 768], FP8, name=f"xar{g}", tag=f"xar{g}")
                nc.sync.dma_start(x_c[:], xar_d[:, 8 * g:8 * (g + 1), :])
                xar_t[g] = x_c
                if g in (0, 2):
                    h = g // 2
                    xb_c = xres.tile([128, 16, 256], FP8, name=f"xbr{h}", tag=f"xbr{h}")
                    nc.sync.dma_start(xb_c[:], xbr_d[:, 16 * h:16 * (h + 1), :])
                    xbr_t[h] = xb_c

            ones = small.tile([128, 1], BF16)
            nc.vector.memset(ones[:], 1.0)
            quarter = small.tile([128, 1], BF16)
            nc.vector.memset(quarter[:], 0.25)
            onesf = small.tile([128, 1], F32)
            nc.vector.memset(onesf[:], 1.0)

            pk = small.tile([1, 1024], F32)

            # ---- norm chains: nb then na (PE + ScalarE squares) ----
            nb_ps = pse.tile([1, 256], F32, name="nb_ps", tag="pse")
            for g in range(2):
                x2b = x2p.tile([128, 16, 256], BF16, name="x2b", tag="x2b")
                nc.scalar.square(x2b[:], xbn_t[g][:])
                for i in range(16):
                    kt = 16 * g + i
                    nc.tensor.matmul(nb_ps[:], ones[:], x2b[:, i, :],
                                     start=(kt == 0), stop=(kt == KT - 1))
            nc.vector.tensor_copy(pk[0:1, 256:512], nb_ps[:])

            na_ps = pse.tile([1, 256], F32, name="na_ps", tag="pse")
            for g in range(2):
                x2a = x2p.tile([128, 16, 256], BF16, name="x2a", tag="x2a")
                nc.scalar.square(x2a[:], xan_t[g][:])
                for i in range(16):
                    kt = 16 * g + i
                    nc.tensor.matmul(na_ps[:], quarter[:], x2a[:, i, :],
                                     start=(kt == 0), stop=(kt == KT - 1))
            nc.vector.tensor_copy(pk[0:1, 0:256], na_ps[:])

            # ---- fc accumulation on DVE (fp16 in, f32 acc) ----
            acc = small.tile([128, 512], F32)
            red = small.tile([128, 512], F32)
            for oc in range(OC):
                dst = acc if oc == 0 else red
                nc.vector.tensor_reduce(dst[:], fct[oc][:],
                                        axis=mybir.AxisListType.X, op=ALU.add)
                if oc:
                    nc.vector.scalar_tensor_tensor(
                        acc[:], acc[:], 0.0, red[:],
                        op0=ALU.bypass, op1=ALU.add)

            # ---- early main mm: m0/m1 x xbn half (data ready first) ----
            ps7 = psmain.tile([128, 7, 512], F32, name="ps7", tag="ps7")

            def lhs(m, kk):
                if m < 2:
                    g, s = divmod(kk, 8)
                    return xan_t[g][:, 2 * s:2 * s + 2, 128 * m:128 * (m + 1)]
                g, s = divmod(kk, 4)
                return xar_t[g][:, 2 * s:2 * s + 2, 128 * (m - 2):128 * (m - 1)]

            def rhsn(kk):
                g, s = divmod(kk, 8)
                return xbn_t[g][:, 2 * s:2 * s + 2, :]

            def rhsr(kk):
                g, s = divmod(kk, 8)
                return xbr_t[g][:, 2 * s:2 * s + 2, :]

            for kk in range(KK):
                for m in (0, 1):
                    nc.tensor.matmul(ps7[:, m, 0:256], lhs(m, kk), rhsn(kk),
                                     start=(kk == 0), stop=(kk == KK - 1),
                                     perf_mode=DR)

            # ---- w partition-reduce (PE stalls here until fc acc done) ----
            w_ps = pse.tile([1, 512], F32, name="w_ps", tag="pse")
            nc.tensor.matmul(w_ps[:], onesf[:], acc[:], start=True, stop=True)
            nc.vector.tensor_copy(pk[0:1, 512:1024], w_ps[:])

            # ---- collective: pack -> AllGather (gpsimd) ----
            nc.gpsimd.dma_start(pk_in[:], pk[:])
            nc.gpsimd.collective_compute(
                "AllGather", ALU.bypass, replica_groups=grp,
                ins=[pk_in[:]], outs=[pk_sh[:]])
            tok = small.tile([1, 1], F32)
            nc.gpsimd.memset(tok[:], 1.0)

            # ---- rest of main mm ----
            for kk in range(KK):
                for m in range(2, 7):
                    nc.tensor.matmul(ps7[:, m, 0:256], lhs(m, kk), rhsn(kk),
                                     start=(kk == 0), stop=(kk == KK - 1),
                                     perf_mode=DR)
            for kk in range(KK):
                for m in range(7):
                    nc.tensor.matmul(ps7[:, m, 256:512], lhs(m, kk), rhsr(kk),
                                     start=(kk == 0), stop=(kk == KK - 1),
                                     perf_mode=DR)
            ps7b = pse.tile([128, 512], F32, name="ps7b", tag="pse")
            for kk in range(KK):
                nc.tensor.matmul(ps7b[:, 0:256], lhs(7, kk), rhsn(kk),
                                 start=(kk == 0), stop=(kk == KK - 1),
                                 perf_mode=DR)
            for kk in range(KK):
                nc.tensor.matmul(ps7b[:, 256:512], lhs(7, kk), rhsr(kk),
                                 start=(kk == 0), stop=(kk == KK - 1),
                                 perf_mode=DR)

            # ---- post-gather reads (scalar HWDGE, dynamic DRAM offsets) ----
            tokd = small.tile([1, 1], F32)
            nc.scalar.copy(tokd[:], tok[:])  # order scalar after AG
            nav_t = small.tile([128, 4, 2], F32)     # [t, m-half]
            wav_t = small.tile([128, 4, 2], F32)
            for t in range(4):
                k_t = r4 | ((cb + t) & 3)
                off = k_t << 10
                nc.scalar.dma_start(
                    nav_t[:, t, :],
                    bass.AP(tensor=pk_sh.tensor, offset=off,
                            ap=[[1, 128], [128, 2]]))
                nc.scalar.dma_start(
                    wav_t[:, t, :],
                    bass.AP(tensor=pk_sh.tensor, offset=off + 512,
                            ap=[[1, 128], [128, 2]]))
            nbbc = small.tile([128, 512], F32)
            wbbc = small.tile([128, 512], F32)
            for h in range(2):
                u = (r1 + h) & 1
                k_nb = cb + (u << 2)
                k_wb = (cb << 1) + u
                nc.scalar.dma_start(
                    nbbc[:, 256 * h:256 * (h + 1)],
                    bass.AP(tensor=pk_sh.tensor, offset=(k_nb << 10) + 256,
                            ap=[[0, 128], [1, 256]]))
                nc.scalar.dma_start(
                    wbbc[:, 256 * h:256 * (h + 1)],
                    bass.AP(tensor=pk_sh.tensor, offset=(k_wb << 10) + 768,
                            ap=[[0, 128], [1, 256]]))

            # ---- fused epilogue per m-tile ----
            for m in range(8):
                psm = ps7[:, m, :] if m < 7 else ps7b[:]
                nav = nav_t[:, m // 2, m % 2:m % 2 + 1]   # [128,1] per-partition
                wav = wav_t[:, m // 2, m % 2:m % 2 + 1]
                w2m = w2p.tile([128, 512], F32, name="w2m", tag="w2")
                nc.scalar.activation(w2m[:], wbbc[:],
                                     mybir.ActivationFunctionType.Square,
                                     bias=wav, scale=1.0)
                nc.vector.scalar_tensor_tensor(
                    psm, psm, nav, nbbc[:], op0=ALU.add, op1=ALU.add)
                ot = outp.tile([128, 512], F32, name="ot", tag="ot")
                nc.vector.scalar_tensor_tensor(
                    ot[:], psm, 0.0, w2m[:], op0=ALU.max, op1=ALU.mult)
                nc.sync.dma_start(out_d[128 * m:128 * (m + 1), :], ot[:])

    nc.compile()
    return nc


def _p_major(a, np_dtype):
    """[T, cols] -> [128, T//128, cols]."""
    n = a.shape[0] // 128
    return np.ascontiguousarray(
        a.reshape(n, 128, a.shape[1]).transpose(1, 0, 2).astype(np_dtype))


def _core_geom(d):
    r, cb = d >> 2, d & 3
    rows = 1024 * r + (256 * cb + np.arange(MB)) % 1024
    cols = 512 * cb + (256 * r + np.arange(NBC)) % 512
    return rows, cols


def kernel(x, fc_weight, _trace=False):
    """Full inputs in, full [2048, 2048] scores out."""
    x = np.asarray(x, dtype=np.float32)
    fc = np.asarray(fc_weight, dtype=np.float32)
    xf = x.reshape(T, C)
    xa2 = np.ascontiguousarray(xf[:, 0::2]) * -2.0   # [T, 2048]
    xb = np.ascontiguousarray(xf[:, 1::2])

    if "v2" not in _cache:
        _cache["v2"] = _build()
    ncv = _cache["v2"]

    in_maps = []
    geoms = []
    for d in range(D):
        rows, cols = _core_geom(d)
        geoms.append((rows, cols))
        xa_blk = xa2[:, rows]
        xb_blk = xb[:, cols]
        fcd = fc[:, 512 * d:512 * (d + 1)]
        fcs = np.concatenate([fcd[:, 0::2], fcd[:, 1::2]], axis=1)  # [O, 512]
        fcs = np.ascontiguousarray(
            fcs.reshape(OC, 8, 128, 512).transpose(2, 0, 3, 1)
        ).astype(np.float16)                                        # [128,12,512,8]
        in_maps.append({
            "xan": _p_major(xa_blk[:, :256], NP_FP8),
            "xar": _p_major(xa_blk[:, 256:], NP_FP8),
            "xbn": _p_major(xb_blk[:, :256], NP_FP8),
            "xbr": _p_major(xb_blk[:, 256:], NP_FP8),
            "fcs": fcs,
        })

    res = run_bass_kernel_spmd(ncv, in_maps, core_ids=list(range(D)), trace=_trace)
    out = np.empty((CA, CA), dtype=np.float32)
    for d in range(D):
        rows, cols = geoms[d]
        out[np.ix_(rows, cols)] = res.results[d]["scores"]
    if _trace:
        kernel.last_times = (res.exec_time_ns,)
    return out
